# revision 42
# baseline (speedup 1.0000x reference)
"""Trainium2 Bass kernel for nn_Decoder (mean-pool L=16 + overlap-add step 8).

Math (per (b, c) slice, est = est_source[b, c] of shape [256, 4000]):
  A[g, f]      = (1/16) * sum_{l=0..15} est[16*g + l, f]          g in 0..15
  out[8*s + j] = A[j, s] + A[8+j, s-1]                            s in 0..4000
with A[., -1] = A[., 4000] = 0 at the edges.  Output length 8*4001 = 32008.

Kernel strategy (8 cores, 4 slices each): every output sample is the sum of
32 input values (16 from the low half at frame s, 16 from the high half at
frame s-1).  The host packs those 32 addends *m-major* per chunk (addend m
of output k at column m*K + k; pure gather/layout -- the 1/16 is folded
into the fp32->fp16 cast as an exact exponent shift), with the partition
dim mapping to contiguous blocks of output samples.  On device each chunk
is then a binary reduction tree of five tensor_tensor adds, each summing
the first contiguous half-block with the second:

  [128, 4096] f16 -> 2048 -> 1024 -> 512 -> 256 -> [128, 128] f32

Levels 1-4 keep f16 operands packed (DVE 2x mode, 0.5 cyc/elem); the last
level emits f32.  This beats tensor_reduce (no DVE fast modes, 1 cyc/elem)
by ~2x.  No matmul, no PSUM; the only compute engine is the DVE.

DMA structure: loads are chained head-to-tail (each load's first written
SBUF column overlaps the previous chunk's scratch column, a WAW dep) so
chunks complete in sequence at full HBM bandwidth and the reduce tree
pipelines behind the load stream -- 8 concurrent tick-1 DMAs would
otherwise all finish together and serialize all compute after them.
Stores go per chunk on SWDGE (gpsimd): walrus allows only one sync wait
per DMA instruction, and tile adds a queue-ordering wait to HWDGE DMAs
from the 2nd use of each of the 8 queue procs on; with 8 stores on 8
otherwise-idle SWDGE procs each store is tick-1 and carries just its
single DVE data wait.  Each store is [128, 128] f32, 512 B per partition,
contiguous in DRAM.
"""

import sys

if "/opt/trn_rl_repo" not in sys.path:
    sys.path.insert(0, "/opt/trn_rl_repo")

import numpy as np


def _install_ntff_hook():
    """Provide antenv.axon_hooks (absent in this image) so trace=True works.

    The boot-side installer (trn_agent_boot.trn_boot) skips hook setup when
    antenv.axon_hooks is missing; bass_utils then refuses to trace.  We
    register a lazy equivalent backed by the same ctypes NTFF driver.
    """
    import types
    try:
        import antenv
    except ImportError:
        return
    if "antenv.axon_hooks" in sys.modules:
        return
    mod = types.ModuleType("antenv.axon_hooks")
    _state = {}

    def set_axon_ntff_profile_hook(h):
        _state["h"] = h

    def get_axon_ntff_profile_hook():
        if "h" not in _state:
            try:
                from trn_agent_boot.trn_boot import _ntff_profile_via_ctypes
                _state["h"] = _ntff_profile_via_ctypes("/opt/axon/libaxon_pjrt.so")
            except Exception:
                _state["h"] = None
        return _state["h"]

    mod.set_axon_ntff_profile_hook = set_axon_ntff_profile_hook
    mod.get_axon_ntff_profile_hook = get_axon_ntff_profile_hook
    sys.modules["antenv.axon_hooks"] = mod
    antenv.axon_hooks = mod


_install_ntff_hook()

import concourse.bass as bass
import concourse.mybir as mybir
from concourse import tile
from concourse.bass_utils import run_bass_kernel_spmd

# Skip the exit barrier/sem-clear churn (see _drain_and_barrier).
_SLIM_EXIT = True


class _SingleWaitTileContext(tile.TileContext):
    """TileContext whose kernel-tail drain never carries multiple sem waits.

    The pinned walrus build rejects any instruction with more than one sync
    wait ("Too many sync wait commands").  Tile's default exit emits a single
    Drain waiting on every outstanding proc semaphore.  Instead, emit one
    wait_ge per proc on the SP sequencer (each a single-wait instruction),
    then a wait-free drain.
    """

    # proc indices >= _FIRST_DMA_PROC are DMA lanes whose semaphores advance
    # by 16 per op (one inc per SDMA engine) while the vector clock ticks 1.
    _FIRST_DMA_PROC = 11

    def _drain_and_barrier(self, tick_clock, wait_clock):
        nc = self.nc
        clock = tick_clock.global_clock  # bass_rust.VectorClock: 27 ints
        allocated = wait_clock.sems.allocated()
        for proc_idx, tick in enumerate(clock):
            if tick > 0 and proc_idx in allocated:
                val = tick * 16 if proc_idx >= self._FIRST_DMA_PROC else tick
                nc.sync.wait_ge(allocated[proc_idx], val)
        nc.sync.drain()
        if _SLIM_EXIT:
            # Outermost context, program ends right after: skip the
            # barrier + semaphore clear + barrier churn (~4-8 us).  The
            # wait_ge ladder above already gates the SP engine's last
            # instruction on every DMA/engine completion, and each launch
            # re-initializes semaphores in the framework preamble.
            popped = nc._tile_sem_poison_stack.pop()
            assert popped is self._sem_poison
            return
        nc.all_engine_barrier()
        popped = nc._tile_sem_poison_stack.pop()
        assert popped is self._sem_poison
        nc.clear_and_free_semaphores(list(self.sems.allocated().values()))
        nc.all_engine_barrier()

# Problem constants (hardcoded per spec)
B, C, D2, FRAMES = 16, 2, 256, 4000
L = 16
SUB = FRAMES + 1          # 4001 output subframes per slice
OUT_LEN = 8 * SUB         # 32008
N_CORES = 8
SLICES = (B * C) // N_CORES   # 4 slices per core
FPP = 32                  # output frames per partition (128*32 = 4096 >= 4001)
OUTC = 8 * FPP            # 256 f32 output cols per partition per slice
INC = 32 * OUTC           # 8192 f16 input cols per partition per slice

# Variable-width chunks (f16 input cols per partition).  Small chunks at
# the ends: the first tree starts ~2 us earlier, and the trailing
# tree+store after the last DMA completes is half-sized.  Sum per slice =
# INC.  Fields: (slice, col offset within slice, width, engine) where
# engine "G" runs the whole reduce tree on gpsimd (~8 us, hidden under
# the load stream) and "D" on the DVE.  The first two loads are issued
# from the scalar HWDGE queue, whose engine preamble retires ~1.5 us
# before sync's, pulling the whole stream earlier.
CHUNKS = [
    (0, 0, 4096, "D"), (0, 4096, 4096, "D"),
    (1, 0, 4096, "D"), (1, 4096, 4096, "D"),
    (2, 0, 4096, "D"), (2, 4096, 4096, "D"),
    (3, 0, 4096, "D"), (3, 4096, 4096, "D"),
]
NCHT = len(CHUNKS)
# All loads issue on the sync queue: scalar-queue (qActDynamicHW) loads
# measured ~7 us slower to complete -- the DMA engines deprioritize it.
N_SCALAR_ISSUE = 0
# Tree units: chunks whose L3..L5 levels run as one merged instruction
# each (3D APs over the pair) -- saves ~0.45 us per pair of equal-width
# adjacent chunks.  Units must be same-width, same-engine, same store.
# First and last chunks stay solo: the first tree starts as early as
# possible, the last tree's post-stream tail stays short.  (A variant
# with extra 2048-wide chunks at both ends measured ~1.5 us slower:
# the 10-DMA stream runs below full rate.)
TREE_UNITS = [(0,), (1,), (2, 3), (4, 5), (6,), (7,)]
# store units: (slice, col offset in y, width in f32 cols, tree unit idx)
STORES = [
    (0, 0, 128, 0), (0, 128, 128, 1),
    (1, 0, 256, 2),
    (2, 0, 256, 3),
    (3, 0, 128, 4), (3, 128, 128, 5),
]

_CACHE = {}


def _build_nc() -> bass.Bass:
    f16 = mybir.dt.float16
    f32 = mybir.dt.float32

    tot = SLICES * INC
    offs = []
    o = 0
    for _, _, w, _ in CHUNKS:
        offs.append(o)
        o += w
    assert o == tot

    nc = bass.Bass()
    # Host-packed input, one [128, W] f16 block per chunk, split by width
    # into two tensors.  (A flat 1-D tensor with a rearranged AP was
    # measured ~25% slower to stream.)
    chunk_src = {}
    for width, name in ((2048, "xs"), (4096, "xb")):
        cis = [ci for ci, c in enumerate(CHUNKS) if c[2] == width]
        if cis:
            t = nc.dram_tensor(name, [len(cis), 128, width], f16,
                               kind="ExternalInput")
            chunk_src.update({ci: (t, j) for j, ci in enumerate(cis)})
    # y[i, p, k] = output sample 256*p + k of slice i.
    y_d = nc.dram_tensor("y", [SLICES, 128, OUTC], f32, kind="ExternalOutput")

    unit_store = {tu: si for si, (_, _, _, tu) in enumerate(STORES)}

    with _SingleWaitTileContext(nc) as tc:
        with (
            tc.tile_pool(name="xp", bufs=1) as xp,
            tc.tile_pool(name="zp", bufs=NCHT) as zp,
            tc.tile_pool(name="yp", bufs=len(STORES)) as yp,
        ):
            # One big input buffer mirroring the DRAM chunk layout.  Loads
            # are issued with no data deps (the first 8 are tick-1 on the 8
            # HWDGE queue procs); the DMA subsystem drains them with
            # staggered completions, so the reduce trees pipeline behind
            # the load stream.  (Explicit WAW chaining was tried and loses:
            # each completion->next-load handoff costs ~1.7 us of dead
            # stream time.)
            xb = xp.tile([128, tot], f16)
            yos = [yp.tile([128, sw], f32, tag="y", name=f"yo{si}")
                   for si, (_, _, sw, _) in enumerate(STORES)]
            add = mybir.AluOpType.add
            for ui, unit in enumerate(TREE_UNITS):
                n = len(unit)
                W = CHUNKS[unit[0]][2]
                eng = nc.gpsimd if CHUNKS[unit[0]][3] == "G" else nc.vector
                w = W // 2
                # Per-chunk L1 + L2 (each L1 carries the single load wait).
                z2u = zp.tile([128, n * W // 4], f16, tag="z2")
                for j, ci in enumerate(unit):
                    lo = offs[ci]
                    issue = nc.scalar if ci < N_SCALAR_ISSUE else nc.sync
                    src, sj = chunk_src[ci]
                    issue.dma_start(out=xb[:, lo : lo + W], in_=src[sj])
                    z1 = zp.tile([128, w], f16, tag="z1")
                    eng.tensor_tensor(out=z1[:], in0=xb[:, lo : lo + w],
                                      in1=xb[:, lo + w : lo + 2 * w], op=add)
                    eng.tensor_tensor(
                        out=z2u[:, j * (w // 2) : (j + 1) * (w // 2)],
                        in0=z1[:, : w // 2], in1=z1[:, w // 2 :], op=add)
                # Merged L3..L5 over the unit's chunks via 3D APs (the
                # final level emits f32 straight into the store tile).
                si = unit_store[ui]
                yo = yos[si]
                z3u = zp.tile([128, n * W // 8], f16, tag="z3")
                z4u = zp.tile([128, n * W // 16], f16, tag="z4")
                for src, dst, ww in ((z2u, z3u, W // 4), (z3u, z4u, W // 8)):
                    sv = src[:].rearrange("p (c w) -> p c w", w=ww)
                    dv = dst[:].rearrange("p (c w) -> p c w", w=ww // 2)
                    eng.tensor_tensor(out=dv, in0=sv[:, :, : ww // 2],
                                      in1=sv[:, :, ww // 2 :], op=add)
                z4v = z4u[:].rearrange("p (c w) -> p c w", w=W // 16)
                k0 = CHUNKS[unit[0]][1] // 32 - STORES[si][1]
                yv = (yo[:, k0 : k0 + n * (W // 32)]
                      .rearrange("p (c w) -> p c w", w=W // 32))
                eng.tensor_tensor(out=yv, in0=z4v[:, :, : W // 32],
                                  in1=z4v[:, :, W // 32 :], op=add)
                # SWDGE stores: on the otherwise-idle gpsimd DMA procs,
                # each tick-1 so its single sync wait is the merged data
                # dependency of its tree unit's engine.
                for sj, (sl, soff, sw, tu) in enumerate(STORES):
                    if tu == ui:
                        nc.gpsimd.dma_start(
                            out=y_d[sl][:, soff : soff + sw],
                            in_=yos[sj][:])
    return nc


def _get_nc():
    if "nc" not in _CACHE:
        _CACHE["nc"] = _build_nc()
    return _CACHE["nc"]


def _prep_inputs(flat: np.ndarray) -> np.ndarray:
    """Pack [S, 256, 4000] fp32 into per-core chunk-contiguous fp16 blobs.

    Pure gather/layout; the only arithmetic is the fp32->fp16 cast with the
    exact 1/16 exponent shift folded in.
    """
    S = flat.shape[0]
    sc = (flat * np.float32(1.0 / L)).astype(np.float16)
    xl = sc[:, :128, :].reshape(S, 8, 16, FRAMES)
    xh = sc[:, 128:, :].reshape(S, 8, 16, FRAMES)
    # a[slice, s, j, m] = addend m of output sample 8*s + j
    a = np.zeros((S, 128 * FPP, 8, 32), dtype=np.float16)
    a[:, :FRAMES, :, :16] = xl.transpose(0, 3, 1, 2)
    a[:, 1 : FRAMES + 1, :, 16:] = xh.transpose(0, 3, 1, 2)
    # M[i, p, m, k]: addend m of output col k (= 8t+j) of partition p
    m = a.reshape(S, 128, 32, 8, 32).transpose(0, 1, 4, 2, 3)
    m = np.ascontiguousarray(m).reshape(S, 128, 32, OUTC)
    n_cores = S // SLICES
    cores = []
    for k in range(n_cores):
        xs, xbg = [], []
        for sl, off, w, _ in CHUNKS:
            k0 = off // 32
            blk = m[SLICES * k + sl][:, :, k0 : k0 + w // 32]  # [128, 32, K]
            (xs if w == 2048 else xbg).append(blk.reshape(128, w))
        im = {}
        if xs:
            im["xs"] = np.stack(xs)
        if xbg:
            im["xb"] = np.stack(xbg)
        cores.append(im)
    return cores


def kernel(est_source: np.ndarray, _trace: bool = False) -> np.ndarray:
    est = np.ascontiguousarray(np.asarray(est_source), dtype=np.float32)
    assert est.shape == (B, C, D2, FRAMES)
    flat = est.reshape(B * C, D2, FRAMES)
    x = _prep_inputs(flat)

    nc = _get_nc()
    in_maps = [x[k] for k in range(N_CORES)]
    res = run_bass_kernel_spmd(nc, in_maps, core_ids=list(range(N_CORES)),
                               trace=_trace)
    _CACHE["last_results"] = res
    outs = [
        res.results[k]["y"].reshape(SLICES, 128 * OUTC)[:, :OUT_LEN]
        for k in range(N_CORES)
    ]
    return np.concatenate(outs, axis=0).reshape(B, C, OUT_LEN)


# revision 44
# speedup vs baseline: 1.0060x; 1.0060x over previous
"""Trainium2 Bass kernel for nn_Decoder (mean-pool L=16 + overlap-add step 8).

Math (per (b, c) slice, est = est_source[b, c] of shape [256, 4000]):
  A[g, f]      = (1/16) * sum_{l=0..15} est[16*g + l, f]          g in 0..15
  out[8*s + j] = A[j, s] + A[8+j, s-1]                            s in 0..4000
with A[., -1] = A[., 4000] = 0 at the edges.  Output length 8*4001 = 32008.

Kernel strategy (8 cores, 4 slices each): every output sample is the sum of
32 input values (16 from the low half at frame s, 16 from the high half at
frame s-1).  The host packs those 32 addends *m-major* per chunk (addend m
of output k at column m*K + k; pure gather/layout -- the 1/16 is folded
into the fp32->fp16 cast as an exact exponent shift), with the partition
dim mapping to contiguous blocks of output samples.  On device each chunk
is then a binary reduction tree of five tensor_tensor adds, each summing
the first contiguous half-block with the second:

  [128, 4096] f16 -> 2048 -> 1024 -> 512 -> 256 -> [128, 128] f32

Levels 1-4 keep f16 operands packed (DVE 2x mode, 0.5 cyc/elem); the last
level emits f32.  This beats tensor_reduce (no DVE fast modes, 1 cyc/elem)
by ~2x.  No matmul, no PSUM; the only compute engine is the DVE.

DMA structure: loads are chained head-to-tail (each load's first written
SBUF column overlaps the previous chunk's scratch column, a WAW dep) so
chunks complete in sequence at full HBM bandwidth and the reduce tree
pipelines behind the load stream -- 8 concurrent tick-1 DMAs would
otherwise all finish together and serialize all compute after them.
Stores go per chunk on SWDGE (gpsimd): walrus allows only one sync wait
per DMA instruction, and tile adds a queue-ordering wait to HWDGE DMAs
from the 2nd use of each of the 8 queue procs on; with 8 stores on 8
otherwise-idle SWDGE procs each store is tick-1 and carries just its
single DVE data wait.  Each store is [128, 128] f32, 512 B per partition,
contiguous in DRAM.
"""

import sys

if "/opt/trn_rl_repo" not in sys.path:
    sys.path.insert(0, "/opt/trn_rl_repo")

import numpy as np


def _install_ntff_hook():
    """Provide antenv.axon_hooks (absent in this image) so trace=True works.

    The boot-side installer (trn_agent_boot.trn_boot) skips hook setup when
    antenv.axon_hooks is missing; bass_utils then refuses to trace.  We
    register a lazy equivalent backed by the same ctypes NTFF driver.
    """
    import types
    try:
        import antenv
    except ImportError:
        return
    if "antenv.axon_hooks" in sys.modules:
        return
    mod = types.ModuleType("antenv.axon_hooks")
    _state = {}

    def set_axon_ntff_profile_hook(h):
        _state["h"] = h

    def get_axon_ntff_profile_hook():
        if "h" not in _state:
            try:
                from trn_agent_boot.trn_boot import _ntff_profile_via_ctypes
                _state["h"] = _ntff_profile_via_ctypes("/opt/axon/libaxon_pjrt.so")
            except Exception:
                _state["h"] = None
        return _state["h"]

    mod.set_axon_ntff_profile_hook = set_axon_ntff_profile_hook
    mod.get_axon_ntff_profile_hook = get_axon_ntff_profile_hook
    sys.modules["antenv.axon_hooks"] = mod
    antenv.axon_hooks = mod


_install_ntff_hook()

import concourse.bass as bass
import concourse.mybir as mybir
from concourse import tile
from concourse.bass_utils import run_bass_kernel_spmd

# Skip the exit barrier/sem-clear churn (see _drain_and_barrier).
_SLIM_EXIT = True


class _SingleWaitTileContext(tile.TileContext):
    """TileContext whose kernel-tail drain never carries multiple sem waits.

    The pinned walrus build rejects any instruction with more than one sync
    wait ("Too many sync wait commands").  Tile's default exit emits a single
    Drain waiting on every outstanding proc semaphore.  Instead, emit one
    wait_ge per proc on the SP sequencer (each a single-wait instruction),
    then a wait-free drain.
    """

    # proc indices >= _FIRST_DMA_PROC are DMA lanes whose semaphores advance
    # by 16 per op (one inc per SDMA engine) while the vector clock ticks 1.
    _FIRST_DMA_PROC = 11

    def _drain_and_barrier(self, tick_clock, wait_clock):
        nc = self.nc
        clock = tick_clock.global_clock  # bass_rust.VectorClock: 27 ints
        allocated = wait_clock.sems.allocated()
        for proc_idx, tick in enumerate(clock):
            if tick > 0 and proc_idx in allocated:
                val = tick * 16 if proc_idx >= self._FIRST_DMA_PROC else tick
                nc.sync.wait_ge(allocated[proc_idx], val)
        nc.sync.drain()
        if _SLIM_EXIT:
            # Outermost context, program ends right after: skip the
            # barrier + semaphore clear + barrier churn (~4-8 us).  The
            # wait_ge ladder above already gates the SP engine's last
            # instruction on every DMA/engine completion, and each launch
            # re-initializes semaphores in the framework preamble.
            popped = nc._tile_sem_poison_stack.pop()
            assert popped is self._sem_poison
            return
        nc.all_engine_barrier()
        popped = nc._tile_sem_poison_stack.pop()
        assert popped is self._sem_poison
        nc.clear_and_free_semaphores(list(self.sems.allocated().values()))
        nc.all_engine_barrier()

# Problem constants (hardcoded per spec)
B, C, D2, FRAMES = 16, 2, 256, 4000
L = 16
SUB = FRAMES + 1          # 4001 output subframes per slice
OUT_LEN = 8 * SUB         # 32008
N_CORES = 8
SLICES = (B * C) // N_CORES   # 4 slices per core
FPP = 32                  # output frames per partition (128*32 = 4096 >= 4001)
OUTC = 8 * FPP            # 256 f32 output cols per partition per slice
INC = 32 * OUTC           # 8192 f16 input cols per partition per slice

# Variable-width chunks (f16 input cols per partition).  Small chunks at
# the ends: the first tree starts ~2 us earlier, and the trailing
# tree+store after the last DMA completes is half-sized.  Sum per slice =
# INC.  Fields: (slice, col offset within slice, width, engine) where
# engine "G" runs the whole reduce tree on gpsimd (~8 us, hidden under
# the load stream) and "D" on the DVE.  The first two loads are issued
# from the scalar HWDGE queue, whose engine preamble retires ~1.5 us
# before sync's, pulling the whole stream earlier.
CHUNKS = [
    (0, 0, 2048, "D"), (0, 2048, 2048, "D"), (0, 4096, 4096, "D"),
    (1, 0, 4096, "D"), (1, 4096, 4096, "D"),
    (2, 0, 4096, "D"), (2, 4096, 4096, "D"),
    (3, 0, 4096, "D"), (3, 4096, 4096, "D"),
]
NCHT = len(CHUNKS)
# All loads issue on the sync queue: scalar-queue (qActDynamicHW) loads
# measured ~7 us slower to complete -- the DMA engines deprioritize it.
N_SCALAR_ISSUE = 0
# Tree units: chunks whose L3..L5 levels run as one merged instruction
# each (3D APs over the pair) -- saves ~0.45 us per pair of equal-width
# adjacent chunks.  Units must be same-width, same-engine, same store.
# First and last chunks stay solo: the first tree starts as early as
# possible, the last tree's post-stream tail stays short.  (A variant
# with extra 2048-wide chunks at both ends measured ~1.5 us slower:
# the 10-DMA stream runs below full rate.)
TREE_UNITS = [(0, 1), (2,), (3, 4), (5, 6), (7,), (8,)]
# store units: (slice, col offset in y, width in f32 cols, tree unit idx)
STORES = [
    (0, 0, 128, 0), (0, 128, 128, 1),
    (1, 0, 256, 2),
    (2, 0, 256, 3),
    (3, 0, 128, 4), (3, 128, 128, 5),
]

_CACHE = {}


def _build_nc() -> bass.Bass:
    f16 = mybir.dt.float16
    f32 = mybir.dt.float32

    tot = SLICES * INC
    offs = []
    o = 0
    for _, _, w, _ in CHUNKS:
        offs.append(o)
        o += w
    assert o == tot

    nc = bass.Bass()
    # Host-packed input, one [128, W] f16 block per chunk, split by width
    # into two tensors.  (A flat 1-D tensor with a rearranged AP was
    # measured ~25% slower to stream.)
    chunk_src = {}
    for width, name in ((2048, "xs"), (4096, "xb")):
        cis = [ci for ci, c in enumerate(CHUNKS) if c[2] == width]
        if cis:
            t = nc.dram_tensor(name, [len(cis), 128, width], f16,
                               kind="ExternalInput")
            chunk_src.update({ci: (t, j) for j, ci in enumerate(cis)})
    # y[i, p, k] = output sample 256*p + k of slice i.
    y_d = nc.dram_tensor("y", [SLICES, 128, OUTC], f32, kind="ExternalOutput")

    unit_store = {tu: si for si, (_, _, _, tu) in enumerate(STORES)}

    with _SingleWaitTileContext(nc) as tc:
        with (
            tc.tile_pool(name="xp", bufs=1) as xp,
            tc.tile_pool(name="zp", bufs=NCHT) as zp,
            tc.tile_pool(name="yp", bufs=len(STORES)) as yp,
        ):
            # One big input buffer mirroring the DRAM chunk layout.  Loads
            # are issued with no data deps (the first 8 are tick-1 on the 8
            # HWDGE queue procs); the DMA subsystem drains them with
            # staggered completions, so the reduce trees pipeline behind
            # the load stream.  (Explicit WAW chaining was tried and loses:
            # each completion->next-load handoff costs ~1.7 us of dead
            # stream time.)
            xb = xp.tile([128, tot], f16)
            yos = [yp.tile([128, sw], f32, tag="y", name=f"yo{si}")
                   for si, (_, _, sw, _) in enumerate(STORES)]
            add = mybir.AluOpType.add
            for ui, unit in enumerate(TREE_UNITS):
                n = len(unit)
                W = CHUNKS[unit[0]][2]
                eng = nc.gpsimd if CHUNKS[unit[0]][3] == "G" else nc.vector
                w = W // 2
                # Per-chunk L1 + L2 (each L1 carries the single load wait).
                z2u = zp.tile([128, n * W // 4], f16, tag="z2")
                for j, ci in enumerate(unit):
                    lo = offs[ci]
                    issue = nc.scalar if ci < N_SCALAR_ISSUE else nc.sync
                    src, sj = chunk_src[ci]
                    issue.dma_start(out=xb[:, lo : lo + W], in_=src[sj])
                    z1 = zp.tile([128, w], f16, tag="z1")
                    eng.tensor_tensor(out=z1[:], in0=xb[:, lo : lo + w],
                                      in1=xb[:, lo + w : lo + 2 * w], op=add)
                    eng.tensor_tensor(
                        out=z2u[:, j * (w // 2) : (j + 1) * (w // 2)],
                        in0=z1[:, : w // 2], in1=z1[:, w // 2 :], op=add)
                # Merged L3..L5 over the unit's chunks via 3D APs (the
                # final level emits f32 straight into the store tile).
                si = unit_store[ui]
                yo = yos[si]
                z3u = zp.tile([128, n * W // 8], f16, tag="z3")
                z4u = zp.tile([128, n * W // 16], f16, tag="z4")
                for src, dst, ww in ((z2u, z3u, W // 4), (z3u, z4u, W // 8)):
                    sv = src[:].rearrange("p (c w) -> p c w", w=ww)
                    dv = dst[:].rearrange("p (c w) -> p c w", w=ww // 2)
                    eng.tensor_tensor(out=dv, in0=sv[:, :, : ww // 2],
                                      in1=sv[:, :, ww // 2 :], op=add)
                z4v = z4u[:].rearrange("p (c w) -> p c w", w=W // 16)
                k0 = CHUNKS[unit[0]][1] // 32 - STORES[si][1]
                yv = (yo[:, k0 : k0 + n * (W // 32)]
                      .rearrange("p (c w) -> p c w", w=W // 32))
                eng.tensor_tensor(out=yv, in0=z4v[:, :, : W // 32],
                                  in1=z4v[:, :, W // 32 :], op=add)
                # SWDGE stores: on the otherwise-idle gpsimd DMA procs,
                # each tick-1 so its single sync wait is the merged data
                # dependency of its tree unit's engine.
                for sj, (sl, soff, sw, tu) in enumerate(STORES):
                    if tu == ui:
                        nc.gpsimd.dma_start(
                            out=y_d[sl][:, soff : soff + sw],
                            in_=yos[sj][:])
    return nc


def _get_nc():
    if "nc" not in _CACHE:
        _CACHE["nc"] = _build_nc()
    return _CACHE["nc"]


def _prep_inputs(flat: np.ndarray) -> np.ndarray:
    """Pack [S, 256, 4000] fp32 into per-core chunk-contiguous fp16 blobs.

    Pure gather/layout; the only arithmetic is the fp32->fp16 cast with the
    exact 1/16 exponent shift folded in.
    """
    S = flat.shape[0]
    sc = (flat * np.float32(1.0 / L)).astype(np.float16)
    xl = sc[:, :128, :].reshape(S, 8, 16, FRAMES)
    xh = sc[:, 128:, :].reshape(S, 8, 16, FRAMES)
    # a[slice, s, j, m] = addend m of output sample 8*s + j
    a = np.zeros((S, 128 * FPP, 8, 32), dtype=np.float16)
    a[:, :FRAMES, :, :16] = xl.transpose(0, 3, 1, 2)
    a[:, 1 : FRAMES + 1, :, 16:] = xh.transpose(0, 3, 1, 2)
    # M[i, p, m, k]: addend m of output col k (= 8t+j) of partition p
    m = a.reshape(S, 128, 32, 8, 32).transpose(0, 1, 4, 2, 3)
    m = np.ascontiguousarray(m).reshape(S, 128, 32, OUTC)
    n_cores = S // SLICES
    cores = []
    for k in range(n_cores):
        xs, xbg = [], []
        for sl, off, w, _ in CHUNKS:
            k0 = off // 32
            blk = m[SLICES * k + sl][:, :, k0 : k0 + w // 32]  # [128, 32, K]
            (xs if w == 2048 else xbg).append(blk.reshape(128, w))
        im = {}
        if xs:
            im["xs"] = np.stack(xs)
        if xbg:
            im["xb"] = np.stack(xbg)
        cores.append(im)
    return cores


def kernel(est_source: np.ndarray, _trace: bool = False) -> np.ndarray:
    est = np.ascontiguousarray(np.asarray(est_source), dtype=np.float32)
    assert est.shape == (B, C, D2, FRAMES)
    flat = est.reshape(B * C, D2, FRAMES)
    x = _prep_inputs(flat)

    nc = _get_nc()
    in_maps = [x[k] for k in range(N_CORES)]
    res = run_bass_kernel_spmd(nc, in_maps, core_ids=list(range(N_CORES)),
                               trace=_trace)
    _CACHE["last_results"] = res
    outs = [
        res.results[k]["y"].reshape(SLICES, 128 * OUTC)[:, :OUT_LEN]
        for k in range(N_CORES)
    ]
    return np.concatenate(outs, axis=0).reshape(B, C, OUT_LEN)


# revision 47
# speedup vs baseline: 1.0202x; 1.0141x over previous
"""Trainium2 Bass kernel for nn_Decoder (mean-pool L=16 + overlap-add step 8).

Math (per (b, c) slice, est = est_source[b, c] of shape [256, 4000]):
  A[g, f]      = (1/16) * sum_{l=0..15} est[16*g + l, f]          g in 0..15
  out[8*s + j] = A[j, s] + A[8+j, s-1]                            s in 0..4000
with A[., -1] = A[., 4000] = 0 at the edges.  Output length 8*4001 = 32008.

Kernel strategy (8 cores, 4 slices each): every output sample is the sum of
32 input values (16 from the low half at frame s, 16 from the high half at
frame s-1).  The host packs those 32 addends *m-major* per chunk (addend m
of output k at column m*K + k; pure gather/layout -- the 1/16 is folded
into the fp32->fp16 cast as an exact exponent shift), with the partition
dim mapping to contiguous blocks of output samples.  On device each chunk
is then a binary reduction tree of five tensor_tensor adds, each summing
the first contiguous half-block with the second:

  [128, 4096] f16 -> 2048 -> 1024 -> 512 -> 256 -> [128, 128] f32

Levels 1-4 keep f16 operands packed (DVE 2x mode, 0.5 cyc/elem); the last
level emits f32.  This beats tensor_reduce (no DVE fast modes, 1 cyc/elem)
by ~2x.  L3..L5 run merged across chunk pairs (3D APs) where a pair shares
a store, saving per-instruction overhead.  No matmul, no PSUM; the only
compute engine is the DVE (~21 us, pipelined behind ~22 us of loads).

DMA structure: 8 loads of 1 MiB, all issued dependency-free on the sync
HWDGE queue (tick-1 on the 8 tile queue procs).  The DMA subsystem drains
them with ~3 us staggered completions at full aggregate bandwidth
(~380 GB/s), so the trees pipeline behind the stream.  (Variants measured
worse: explicit WAW completion chaining costs ~1.7 us of dead stream per
handoff; scalar-queue loads are drained much later; a flat 1-D input AP
streams ~25% slower; extra small chunks slow the 10-DMA stream.)  Stores
go on SWDGE (gpsimd): walrus allows only one sync wait per DMA
instruction, and tile adds a queue-ordering wait to HWDGE DMAs from the
2nd use of each queue proc on; with 6 stores on the otherwise-idle SWDGE
procs each store is tick-1 and carries just its single merged DVE data
wait.  Each store is 512-1024 B per partition, contiguous in DRAM.

The TileContext exit is slimmed to a per-proc wait ladder + drain
(_SLIM_EXIT): the stock barrier + semaphore-clear + barrier costs ~4 us
more.  A further ~6.7 us epilogue (one EVENT_SEMAPHORE clear per device
semaphore id 2..255, round-robined over the five engines) is emitted by
the compile pipeline for every NEFF and is not reachable from bass.
"""

import sys

if "/opt/trn_rl_repo" not in sys.path:
    sys.path.insert(0, "/opt/trn_rl_repo")

import numpy as np


def _install_ntff_hook():
    """Provide antenv.axon_hooks (absent in this image) so trace=True works.

    The boot-side installer (trn_agent_boot.trn_boot) skips hook setup when
    antenv.axon_hooks is missing; bass_utils then refuses to trace.  We
    register a lazy equivalent backed by the same ctypes NTFF driver.
    """
    import types
    try:
        import antenv
    except ImportError:
        return
    if "antenv.axon_hooks" in sys.modules:
        return
    mod = types.ModuleType("antenv.axon_hooks")
    _state = {}

    def set_axon_ntff_profile_hook(h):
        _state["h"] = h

    def get_axon_ntff_profile_hook():
        if "h" not in _state:
            try:
                from trn_agent_boot.trn_boot import _ntff_profile_via_ctypes
                _state["h"] = _ntff_profile_via_ctypes("/opt/axon/libaxon_pjrt.so")
            except Exception:
                _state["h"] = None
        return _state["h"]

    mod.set_axon_ntff_profile_hook = set_axon_ntff_profile_hook
    mod.get_axon_ntff_profile_hook = get_axon_ntff_profile_hook
    sys.modules["antenv.axon_hooks"] = mod
    antenv.axon_hooks = mod


_install_ntff_hook()

import concourse.bass as bass
import concourse.mybir as mybir
from concourse import tile
from concourse.bass_utils import run_bass_kernel_spmd

# Skip the exit barrier/sem-clear churn (see _drain_and_barrier).
_SLIM_EXIT = True


class _SingleWaitTileContext(tile.TileContext):
    """TileContext whose kernel-tail drain never carries multiple sem waits.

    The pinned walrus build rejects any instruction with more than one sync
    wait ("Too many sync wait commands").  Tile's default exit emits a single
    Drain waiting on every outstanding proc semaphore.  Instead, emit one
    wait_ge per proc on the SP sequencer (each a single-wait instruction),
    then a wait-free drain.
    """

    # proc indices >= _FIRST_DMA_PROC are DMA lanes whose semaphores advance
    # by 16 per op (one inc per SDMA engine) while the vector clock ticks 1.
    _FIRST_DMA_PROC = 11

    def _drain_and_barrier(self, tick_clock, wait_clock):
        nc = self.nc
        clock = tick_clock.global_clock  # bass_rust.VectorClock: 27 ints
        allocated = wait_clock.sems.allocated()
        for proc_idx, tick in enumerate(clock):
            if tick > 0 and proc_idx in allocated:
                val = tick * 16 if proc_idx >= self._FIRST_DMA_PROC else tick
                nc.sync.wait_ge(allocated[proc_idx], val)
        nc.sync.drain()
        if _SLIM_EXIT:
            # Outermost context, program ends right after: skip the
            # barrier + semaphore clear + barrier churn (~4-8 us).  The
            # wait_ge ladder above already gates the SP engine's last
            # instruction on every DMA/engine completion, and each launch
            # re-initializes semaphores in the framework preamble.
            popped = nc._tile_sem_poison_stack.pop()
            assert popped is self._sem_poison
            return
        nc.all_engine_barrier()
        popped = nc._tile_sem_poison_stack.pop()
        assert popped is self._sem_poison
        nc.clear_and_free_semaphores(list(self.sems.allocated().values()))
        nc.all_engine_barrier()

# Problem constants (hardcoded per spec)
B, C, D2, FRAMES = 16, 2, 256, 4000
L = 16
SUB = FRAMES + 1          # 4001 output subframes per slice
OUT_LEN = 8 * SUB         # 32008
N_CORES = 8
SLICES = (B * C) // N_CORES   # 4 slices per core
FPP = 32                  # output frames per partition (128*32 = 4096 >= 4001)
OUTC = 8 * FPP            # 256 f32 output cols per partition per slice
INC = 32 * OUTC           # 8192 f16 input cols per partition per slice

# Variable-width chunks (f16 input cols per partition).  Small chunks at
# the ends: the first tree starts ~2 us earlier, and the trailing
# tree+store after the last DMA completes is half-sized.  Sum per slice =
# INC.  Fields: (slice, col offset within slice, width, engine) where
# engine "G" runs the whole reduce tree on gpsimd (~8 us, hidden under
# the load stream) and "D" on the DVE.  The first two loads are issued
# from the scalar HWDGE queue, whose engine preamble retires ~1.5 us
# before sync's, pulling the whole stream earlier.
CHUNKS = [
    (0, 0, 4096, "D"), (0, 4096, 4096, "D"),
    (1, 0, 4096, "D"), (1, 4096, 4096, "D"),
    (2, 0, 4096, "D"), (2, 4096, 4096, "D"),
    (3, 0, 4096, "D"), (3, 4096, 4096, "D"),
]
NCHT = len(CHUNKS)
# All loads issue on the sync queue: scalar-queue (qActDynamicHW) loads
# measured ~7 us slower to complete -- the DMA engines deprioritize it.
N_SCALAR_ISSUE = 0
# Tree units: chunks whose L3..L5 levels run as one merged instruction
# each (3D APs over the pair) -- saves ~0.45 us per pair of equal-width
# adjacent chunks.  Units must be same-width, same-engine, same store.
# First and last chunks stay solo: the first tree starts as early as
# possible, the last tree's post-stream tail stays short.  (A variant
# with extra 2048-wide chunks at both ends measured ~1.5 us slower:
# the 10-DMA stream runs below full rate.)
TREE_UNITS = [(0,), (1,), (2, 3), (4, 5), (6,), (7,)]
# store units: (slice, col offset in y, width in f32 cols, tree unit idx)
STORES = [
    (0, 0, 128, 0), (0, 128, 128, 1),
    (1, 0, 256, 2),
    (2, 0, 256, 3),
    (3, 0, 128, 4), (3, 128, 128, 5),
]

_CACHE = {}


def _build_nc() -> bass.Bass:
    f16 = mybir.dt.float16
    f32 = mybir.dt.float32

    tot = SLICES * INC
    offs = []
    o = 0
    for _, _, w, _ in CHUNKS:
        offs.append(o)
        o += w
    assert o == tot

    nc = bass.Bass()
    # Host-packed input, one [128, W] f16 block per chunk, split by width
    # into two tensors.  (A flat 1-D tensor with a rearranged AP was
    # measured ~25% slower to stream.)
    chunk_src = {}
    for width, name in ((2048, "xs"), (4096, "xb")):
        cis = [ci for ci, c in enumerate(CHUNKS) if c[2] == width]
        if cis:
            t = nc.dram_tensor(name, [len(cis), 128, width], f16,
                               kind="ExternalInput")
            chunk_src.update({ci: (t, j) for j, ci in enumerate(cis)})
    # y[i, p, k] = output sample 256*p + k of slice i.
    y_d = nc.dram_tensor("y", [SLICES, 128, OUTC], f32, kind="ExternalOutput")

    unit_store = {tu: si for si, (_, _, _, tu) in enumerate(STORES)}

    with _SingleWaitTileContext(nc) as tc:
        with (
            tc.tile_pool(name="xp", bufs=1) as xp,
            tc.tile_pool(name="zp", bufs=NCHT) as zp,
            tc.tile_pool(name="yp", bufs=len(STORES)) as yp,
        ):
            # One big input buffer mirroring the DRAM chunk layout.  Loads
            # are issued with no data deps (the first 8 are tick-1 on the 8
            # HWDGE queue procs); the DMA subsystem drains them with
            # staggered completions, so the reduce trees pipeline behind
            # the load stream.  (Explicit WAW chaining was tried and loses:
            # each completion->next-load handoff costs ~1.7 us of dead
            # stream time.)
            xb = xp.tile([128, tot], f16)
            yos = [yp.tile([128, sw], f32, tag="y", name=f"yo{si}")
                   for si, (_, _, sw, _) in enumerate(STORES)]
            add = mybir.AluOpType.add
            for ui, unit in enumerate(TREE_UNITS):
                n = len(unit)
                W = CHUNKS[unit[0]][2]
                eng = nc.gpsimd if CHUNKS[unit[0]][3] == "G" else nc.vector
                w = W // 2
                # Per-chunk L1 + L2 (each L1 carries the single load wait).
                z2u = zp.tile([128, n * W // 4], f16, tag="z2")
                for j, ci in enumerate(unit):
                    lo = offs[ci]
                    issue = nc.scalar if ci < N_SCALAR_ISSUE else nc.sync
                    src, sj = chunk_src[ci]
                    issue.dma_start(out=xb[:, lo : lo + W], in_=src[sj])
                    z1 = zp.tile([128, w], f16, tag="z1")
                    eng.tensor_tensor(out=z1[:], in0=xb[:, lo : lo + w],
                                      in1=xb[:, lo + w : lo + 2 * w], op=add)
                    eng.tensor_tensor(
                        out=z2u[:, j * (w // 2) : (j + 1) * (w // 2)],
                        in0=z1[:, : w // 2], in1=z1[:, w // 2 :], op=add)
                # Merged L3..L5 over the unit's chunks via 3D APs (the
                # final level emits f32 straight into the store tile).
                si = unit_store[ui]
                yo = yos[si]
                z3u = zp.tile([128, n * W // 8], f16, tag="z3")
                z4u = zp.tile([128, n * W // 16], f16, tag="z4")
                for src, dst, ww in ((z2u, z3u, W // 4), (z3u, z4u, W // 8)):
                    sv = src[:].rearrange("p (c w) -> p c w", w=ww)
                    dv = dst[:].rearrange("p (c w) -> p c w", w=ww // 2)
                    eng.tensor_tensor(out=dv, in0=sv[:, :, : ww // 2],
                                      in1=sv[:, :, ww // 2 :], op=add)
                z4v = z4u[:].rearrange("p (c w) -> p c w", w=W // 16)
                k0 = CHUNKS[unit[0]][1] // 32 - STORES[si][1]
                yv = (yo[:, k0 : k0 + n * (W // 32)]
                      .rearrange("p (c w) -> p c w", w=W // 32))
                eng.tensor_tensor(out=yv, in0=z4v[:, :, : W // 32],
                                  in1=z4v[:, :, W // 32 :], op=add)
                # SWDGE stores: on the otherwise-idle gpsimd DMA procs,
                # each tick-1 so its single sync wait is the merged data
                # dependency of its tree unit's engine.
                for sj, (sl, soff, sw, tu) in enumerate(STORES):
                    if tu == ui:
                        nc.gpsimd.dma_start(
                            out=y_d[sl][:, soff : soff + sw],
                            in_=yos[sj][:])
    return nc


def _get_nc():
    if "nc" not in _CACHE:
        _CACHE["nc"] = _build_nc()
    return _CACHE["nc"]


def _prep_inputs(flat: np.ndarray) -> np.ndarray:
    """Pack [S, 256, 4000] fp32 into per-core chunk-contiguous fp16 blobs.

    Pure gather/layout; the only arithmetic is the fp32->fp16 cast with the
    exact 1/16 exponent shift folded in.
    """
    S = flat.shape[0]
    sc = (flat * np.float32(1.0 / L)).astype(np.float16)
    xl = sc[:, :128, :].reshape(S, 8, 16, FRAMES)
    xh = sc[:, 128:, :].reshape(S, 8, 16, FRAMES)
    # a[slice, s, j, m] = addend m of output sample 8*s + j
    a = np.zeros((S, 128 * FPP, 8, 32), dtype=np.float16)
    a[:, :FRAMES, :, :16] = xl.transpose(0, 3, 1, 2)
    a[:, 1 : FRAMES + 1, :, 16:] = xh.transpose(0, 3, 1, 2)
    # M[i, p, m, k]: addend m of output col k (= 8t+j) of partition p
    m = a.reshape(S, 128, 32, 8, 32).transpose(0, 1, 4, 2, 3)
    m = np.ascontiguousarray(m).reshape(S, 128, 32, OUTC)
    n_cores = S // SLICES
    cores = []
    for k in range(n_cores):
        xs, xbg = [], []
        for sl, off, w, _ in CHUNKS:
            k0 = off // 32
            blk = m[SLICES * k + sl][:, :, k0 : k0 + w // 32]  # [128, 32, K]
            (xs if w == 2048 else xbg).append(blk.reshape(128, w))
        im = {}
        if xs:
            im["xs"] = np.stack(xs)
        if xbg:
            im["xb"] = np.stack(xbg)
        cores.append(im)
    return cores


def kernel(est_source: np.ndarray, _trace: bool = False) -> np.ndarray:
    est = np.ascontiguousarray(np.asarray(est_source), dtype=np.float32)
    assert est.shape == (B, C, D2, FRAMES)
    flat = est.reshape(B * C, D2, FRAMES)
    x = _prep_inputs(flat)

    nc = _get_nc()
    in_maps = [x[k] for k in range(N_CORES)]
    res = run_bass_kernel_spmd(nc, in_maps, core_ids=list(range(N_CORES)),
                               trace=_trace)
    _CACHE["last_results"] = res
    outs = [
        res.results[k]["y"].reshape(SLICES, 128 * OUTC)[:, :OUT_LEN]
        for k in range(N_CORES)
    ]
    return np.concatenate(outs, axis=0).reshape(B, C, OUT_LEN)


# revision 48
# speedup vs baseline: 1.0810x; 1.0596x over previous
"""Trainium2 Bass kernel for nn_Decoder (mean-pool L=16 + overlap-add step 8).

Math (per (b, c) slice, est = est_source[b, c] of shape [256, 4000]):
  A[g, f]      = (1/16) * sum_{l=0..15} est[16*g + l, f]          g in 0..15
  out[8*s + j] = A[j, s] + A[8+j, s-1]                            s in 0..4000
with A[., -1] = A[., 4000] = 0 at the edges.  Output length 8*4001 = 32008.

Kernel strategy (8 cores, 4 slices each): every output sample is the sum of
32 input values (16 from the low half at frame s, 16 from the high half at
frame s-1).  The host packs those 32 addends *m-major* per chunk (addend m
of output k at column m*K + k; pure gather/layout -- the 1/16 is folded
into the fp32->fp16 cast as an exact exponent shift), with the partition
dim mapping to contiguous blocks of output samples.  On device each chunk
is then a binary reduction tree of five tensor_tensor adds, each summing
the first contiguous half-block with the second:

  [128, 4096] f16 -> 2048 -> 1024 -> 512 -> 256 -> [128, 128] f32

Levels 1-4 keep f16 operands packed (DVE 2x mode, 0.5 cyc/elem); the last
level emits f32.  This beats tensor_reduce (no DVE fast modes, 1 cyc/elem)
by ~2x.  L3..L5 run merged across chunk pairs (3D APs) where a pair shares
a store, saving per-instruction overhead.  No matmul, no PSUM; the only
compute engine is the DVE (~21 us, pipelined behind ~22 us of loads).

DMA structure: 8 loads of 1 MiB, all issued dependency-free on the sync
HWDGE queue (tick-1 on the 8 tile queue procs).  The DMA subsystem drains
them with ~3 us staggered completions at full aggregate bandwidth
(~380 GB/s), so the trees pipeline behind the stream.  (Variants measured
worse: explicit WAW completion chaining costs ~1.7 us of dead stream per
handoff; scalar-queue loads are drained much later; a flat 1-D input AP
streams ~25% slower; extra small chunks slow the 10-DMA stream.)  Stores
go on SWDGE (gpsimd): walrus allows only one sync wait per DMA
instruction, and tile adds a queue-ordering wait to HWDGE DMAs from the
2nd use of each queue proc on; with 6 stores on the otherwise-idle SWDGE
procs each store is tick-1 and carries just its single merged DVE data
wait.  Each store is 512-1024 B per partition, contiguous in DRAM.

The TileContext exit is slimmed to a per-proc wait ladder + drain
(_SLIM_EXIT): the stock barrier + semaphore-clear + barrier costs ~4 us
more.  A further ~6.7 us epilogue (one EVENT_SEMAPHORE clear per device
semaphore id 2..255, round-robined over the five engines) is emitted by
the compile pipeline for every NEFF and is not reachable from bass.
"""

import sys

if "/opt/trn_rl_repo" not in sys.path:
    sys.path.insert(0, "/opt/trn_rl_repo")

import numpy as np


def _install_ntff_hook():
    """Provide antenv.axon_hooks (absent in this image) so trace=True works.

    The boot-side installer (trn_agent_boot.trn_boot) skips hook setup when
    antenv.axon_hooks is missing; bass_utils then refuses to trace.  We
    register a lazy equivalent backed by the same ctypes NTFF driver.
    """
    import types
    try:
        import antenv
    except ImportError:
        return
    if "antenv.axon_hooks" in sys.modules:
        return
    mod = types.ModuleType("antenv.axon_hooks")
    _state = {}

    def set_axon_ntff_profile_hook(h):
        _state["h"] = h

    def get_axon_ntff_profile_hook():
        if "h" not in _state:
            try:
                from trn_agent_boot.trn_boot import _ntff_profile_via_ctypes
                _state["h"] = _ntff_profile_via_ctypes("/opt/axon/libaxon_pjrt.so")
            except Exception:
                _state["h"] = None
        return _state["h"]

    mod.set_axon_ntff_profile_hook = set_axon_ntff_profile_hook
    mod.get_axon_ntff_profile_hook = get_axon_ntff_profile_hook
    sys.modules["antenv.axon_hooks"] = mod
    antenv.axon_hooks = mod


_install_ntff_hook()

import concourse.bass as bass
import concourse.mybir as mybir
from concourse import tile
from concourse.bass_utils import run_bass_kernel_spmd

# Skip the exit barrier/sem-clear churn (see _drain_and_barrier).
_SLIM_EXIT = True


class _SingleWaitTileContext(tile.TileContext):
    """TileContext whose kernel-tail drain never carries multiple sem waits.

    The pinned walrus build rejects any instruction with more than one sync
    wait ("Too many sync wait commands").  Tile's default exit emits a single
    Drain waiting on every outstanding proc semaphore.  Instead, emit one
    wait_ge per proc on the SP sequencer (each a single-wait instruction),
    then a wait-free drain.
    """

    # proc indices >= _FIRST_DMA_PROC are DMA lanes whose semaphores advance
    # by 16 per op (one inc per SDMA engine) while the vector clock ticks 1.
    _FIRST_DMA_PROC = 11

    def _drain_and_barrier(self, tick_clock, wait_clock):
        nc = self.nc
        clock = tick_clock.global_clock  # bass_rust.VectorClock: 27 ints
        allocated = wait_clock.sems.allocated()
        for proc_idx, tick in enumerate(clock):
            if tick > 0 and proc_idx in allocated:
                val = tick * 16 if proc_idx >= self._FIRST_DMA_PROC else tick
                nc.sync.wait_ge(allocated[proc_idx], val)
        nc.sync.drain()
        if _SLIM_EXIT:
            # Outermost context, program ends right after: skip the
            # barrier + semaphore clear + barrier churn (~4-8 us).  The
            # wait_ge ladder above already gates the SP engine's last
            # instruction on every DMA/engine completion, and each launch
            # re-initializes semaphores in the framework preamble.
            popped = nc._tile_sem_poison_stack.pop()
            assert popped is self._sem_poison
            return
        nc.all_engine_barrier()
        popped = nc._tile_sem_poison_stack.pop()
        assert popped is self._sem_poison
        nc.clear_and_free_semaphores(list(self.sems.allocated().values()))
        nc.all_engine_barrier()

# Problem constants (hardcoded per spec)
B, C, D2, FRAMES = 16, 2, 256, 4000
L = 16
SUB = FRAMES + 1          # 4001 output subframes per slice
OUT_LEN = 8 * SUB         # 32008
N_CORES = 8
SLICES = (B * C) // N_CORES   # 4 slices per core
FPP = 32                  # output frames per partition (128*32 = 4096 >= 4001)
OUTC = 8 * FPP            # 256 f32 output cols per partition per slice
INC = 32 * OUTC           # 8192 f16 input cols per partition per slice

# Chunks (f16 input cols per partition); widths sum to INC per slice.
# Fields: (slice, col offset within slice, width, engine) where engine
# "D" runs the reduce tree on the DVE ("G" = gpsimd is supported but
# measured slower end-to-end).  Uniform 4096-wide chunks measured best:
# smaller chunks pay per-instruction overhead and slow the DMA stream.
CHUNKS = [
    (0, 0, 4096, "D"), (0, 4096, 4096, "D"),
    (1, 0, 4096, "D"), (1, 4096, 4096, "D"),
    (2, 0, 4096, "D"), (2, 4096, 4096, "D"),
    (3, 0, 4096, "D"), (3, 4096, 4096, "D"),
]
NCHT = len(CHUNKS)
# All loads issue on the sync queue: scalar-queue (qActDynamicHW) loads
# measured ~7 us slower to complete -- the DMA engines deprioritize it.
N_SCALAR_ISSUE = 0
# Tree units: chunks whose L3..L5 levels run as one merged instruction
# each (3D APs over the pair) -- saves ~0.45 us per pair of equal-width
# adjacent chunks.  Units must be same-width, same-engine, same store.
# First and last chunks stay solo: the first tree starts as early as
# possible, the last tree's post-stream tail stays short.  (A variant
# with extra 2048-wide chunks at both ends measured ~1.5 us slower:
# the 10-DMA stream runs below full rate.)
TREE_UNITS = [(0,), (1,), (2, 3), (4, 5), (6,), (7,)]
# store units: (slice, col offset in y, width in f32 cols, tree unit idx)
STORES = [
    (0, 0, 128, 0), (0, 128, 128, 1),
    (1, 0, 256, 2),
    (2, 0, 256, 3),
    (3, 0, 128, 4), (3, 128, 128, 5),
]

_CACHE = {}


def _build_nc() -> bass.Bass:
    f16 = mybir.dt.float16
    f32 = mybir.dt.float32

    tot = SLICES * INC
    offs = []
    o = 0
    for _, _, w, _ in CHUNKS:
        offs.append(o)
        o += w
    assert o == tot

    nc = bass.Bass()
    # Host-packed input, one [128, W] f16 block per chunk, split by width
    # into two tensors.  (A flat 1-D tensor with a rearranged AP was
    # measured ~25% slower to stream.)
    chunk_src = {}
    for width, name in ((2048, "xs"), (4096, "xb")):
        cis = [ci for ci, c in enumerate(CHUNKS) if c[2] == width]
        if cis:
            t = nc.dram_tensor(name, [len(cis), 128, width], f16,
                               kind="ExternalInput")
            chunk_src.update({ci: (t, j) for j, ci in enumerate(cis)})
    # y[i, p, k] = output sample 256*p + k of slice i.
    y_d = nc.dram_tensor("y", [SLICES, 128, OUTC], f32, kind="ExternalOutput")

    unit_store = {tu: si for si, (_, _, _, tu) in enumerate(STORES)}

    with _SingleWaitTileContext(nc) as tc:
        with (
            tc.tile_pool(name="xp", bufs=1) as xp,
            tc.tile_pool(name="zp", bufs=NCHT) as zp,
            tc.tile_pool(name="yp", bufs=len(STORES)) as yp,
        ):
            # One big input buffer mirroring the DRAM chunk layout.  Loads
            # are issued with no data deps (the first 8 are tick-1 on the 8
            # HWDGE queue procs); the DMA subsystem drains them with
            # staggered completions, so the reduce trees pipeline behind
            # the load stream.  (Explicit WAW chaining was tried and loses:
            # each completion->next-load handoff costs ~1.7 us of dead
            # stream time.)
            xb = xp.tile([128, tot], f16)
            yos = [yp.tile([128, sw], f32, tag="y", name=f"yo{si}")
                   for si, (_, _, sw, _) in enumerate(STORES)]
            add = mybir.AluOpType.add
            for ui, unit in enumerate(TREE_UNITS):
                n = len(unit)
                W = CHUNKS[unit[0]][2]
                eng = nc.gpsimd if CHUNKS[unit[0]][3] == "G" else nc.vector
                w = W // 2
                # Per-chunk L1 + L2 (each L1 carries the single load wait).
                z2u = zp.tile([128, n * W // 4], f16, tag="z2")
                for j, ci in enumerate(unit):
                    lo = offs[ci]
                    issue = nc.scalar if ci < N_SCALAR_ISSUE else nc.sync
                    src, sj = chunk_src[ci]
                    issue.dma_start(out=xb[:, lo : lo + W], in_=src[sj])
                    z1 = zp.tile([128, w], f16, tag="z1")
                    eng.tensor_tensor(out=z1[:], in0=xb[:, lo : lo + w],
                                      in1=xb[:, lo + w : lo + 2 * w], op=add)
                    eng.tensor_tensor(
                        out=z2u[:, j * (w // 2) : (j + 1) * (w // 2)],
                        in0=z1[:, : w // 2], in1=z1[:, w // 2 :], op=add)
                # Merged L3..L5 over the unit's chunks via 3D APs (the
                # final level emits f32 straight into the store tile).
                si = unit_store[ui]
                yo = yos[si]
                z3u = zp.tile([128, n * W // 8], f16, tag="z3")
                z4u = zp.tile([128, n * W // 16], f16, tag="z4")
                for src, dst, ww in ((z2u, z3u, W // 4), (z3u, z4u, W // 8)):
                    sv = src[:].rearrange("p (c w) -> p c w", w=ww)
                    dv = dst[:].rearrange("p (c w) -> p c w", w=ww // 2)
                    eng.tensor_tensor(out=dv, in0=sv[:, :, : ww // 2],
                                      in1=sv[:, :, ww // 2 :], op=add)
                z4v = z4u[:].rearrange("p (c w) -> p c w", w=W // 16)
                k0 = CHUNKS[unit[0]][1] // 32 - STORES[si][1]
                yv = (yo[:, k0 : k0 + n * (W // 32)]
                      .rearrange("p (c w) -> p c w", w=W // 32))
                eng.tensor_tensor(out=yv, in0=z4v[:, :, : W // 32],
                                  in1=z4v[:, :, W // 32 :], op=add)
                # SWDGE stores: on the otherwise-idle gpsimd DMA procs,
                # each tick-1 so its single sync wait is the merged data
                # dependency of its tree unit's engine.
                for sj, (sl, soff, sw, tu) in enumerate(STORES):
                    if tu == ui:
                        nc.gpsimd.dma_start(
                            out=y_d[sl][:, soff : soff + sw],
                            in_=yos[sj][:])
    return nc


def _get_nc():
    if "nc" not in _CACHE:
        _CACHE["nc"] = _build_nc()
    return _CACHE["nc"]


def _prep_inputs(flat: np.ndarray) -> np.ndarray:
    """Pack [S, 256, 4000] fp32 into per-core chunk-contiguous fp16 blobs.

    Pure gather/layout; the only arithmetic is the fp32->fp16 cast with the
    exact 1/16 exponent shift folded in.
    """
    S = flat.shape[0]
    sc = (flat * np.float32(1.0 / L)).astype(np.float16)
    xl = sc[:, :128, :].reshape(S, 8, 16, FRAMES)
    xh = sc[:, 128:, :].reshape(S, 8, 16, FRAMES)
    # a[slice, s, j, m] = addend m of output sample 8*s + j
    a = np.zeros((S, 128 * FPP, 8, 32), dtype=np.float16)
    a[:, :FRAMES, :, :16] = xl.transpose(0, 3, 1, 2)
    a[:, 1 : FRAMES + 1, :, 16:] = xh.transpose(0, 3, 1, 2)
    # M[i, p, m, k]: addend m of output col k (= 8t+j) of partition p
    m = a.reshape(S, 128, 32, 8, 32).transpose(0, 1, 4, 2, 3)
    m = np.ascontiguousarray(m).reshape(S, 128, 32, OUTC)
    n_cores = S // SLICES
    cores = []
    for k in range(n_cores):
        xs, xbg = [], []
        for sl, off, w, _ in CHUNKS:
            k0 = off // 32
            blk = m[SLICES * k + sl][:, :, k0 : k0 + w // 32]  # [128, 32, K]
            (xs if w == 2048 else xbg).append(blk.reshape(128, w))
        im = {}
        if xs:
            im["xs"] = np.stack(xs)
        if xbg:
            im["xb"] = np.stack(xbg)
        cores.append(im)
    return cores


def kernel(est_source: np.ndarray, _trace: bool = False) -> np.ndarray:
    est = np.ascontiguousarray(np.asarray(est_source), dtype=np.float32)
    assert est.shape == (B, C, D2, FRAMES)
    flat = est.reshape(B * C, D2, FRAMES)
    x = _prep_inputs(flat)

    nc = _get_nc()
    in_maps = [x[k] for k in range(N_CORES)]
    res = run_bass_kernel_spmd(nc, in_maps, core_ids=list(range(N_CORES)),
                               trace=_trace)
    _CACHE["last_results"] = res
    outs = [
        res.results[k]["y"].reshape(SLICES, 128 * OUTC)[:, :OUT_LEN]
        for k in range(N_CORES)
    ]
    return np.concatenate(outs, axis=0).reshape(B, C, OUT_LEN)
